# revision 47
# baseline (speedup 1.0000x reference)
# DCN CrossLayer kernel for Trainium2 (8 NeuronCores, data-parallel over batch).
#
# Reference computation (per example row x of length D, L=3 layers):
#   cross = x
#   for i in range(L):
#       s_i   = <cross, W_i>                  (scalar per example)
#       cross = x * s_i + bias_i + cross
#
# Algebraic collapse: cross_i = a_i * x + B_i with per-example scalar a_i and
# batch-independent vector B_i = sum_{j<i} bias_j.  Then
#   s_i     = a_i * t_i + c_i,   t_i = <x, W_i>,  c_i = <B_i, W_i>
#   a_{i+1} = a_i * (1 + t_i) + c_i
#   out     = a_L * x + B_L
# so the device kernel only needs the three dot products t_i = <x, W_i>
# (one skinny matmul against W^T), a tiny per-row recurrence, and one
# per-row scale of x.  c_i and B_L are computed on the host (they do not
# depend on the batch).
#
# The kernel is HBM-bandwidth-bound (~400 GB/s aggregate per core across the
# 16 SDMA engines).  To halve the traffic the device I/O is fp16: the host
# casts x -> f16 before upload and upcasts y f16 -> f32 after download.  The
# dot products already ran in f16 on the PE (error ~5e-4, gate is 2e-2).
#
# Device plan per core (2048 rows of 1024, f16):
#   - rows mapped p-major (row = p*TILES + t) so each partition's DMA run is
#     su contiguous rows (8 KiB at su=4) instead of one row (2 KiB)
#   - DMA x in supertiles [128 part, 4, 1024] f16 on the SP HWDGE ring
#   - PE transposes each [128,128] block of x -> PSUM, ACT copies to SBUF
#   - PE matmuls xt_k^T @ Wt_k accumulating t [128 rows, 3] in PSUM
#   - DVE: a3 = ((1+t0)(1+t1)+c1)(1+t2)+c2 ; y = x * a3 (+ B_L), all f16
#   - DMA y out on the ACT HWDGE ring so it can't FIFO-block in-DMAs
import os
from contextlib import ExitStack

import numpy as np

import concourse.bacc as bacc
import concourse.bass as bass
import concourse.tile as tile
from concourse import mybir
from concourse.bass_utils import run_bass_kernel_spmd
from concourse.masks import make_identity

B, D, L = 16384, 1024, 3
N_CORES = 8
ROWS = B // N_CORES  # rows per core
P = 128
TILES = ROWS // P  # 16 row-tiles per core
SUPER = 4  # row-tiles per supertile (1 MiB f16 DMA)
SCHED = [SUPER] * (TILES // SUPER)
KCH = D // P  # 8 d-chunks of 128

F32 = mybir.dt.float32
F16 = mybir.dt.float16
U32 = mybir.dt.uint32

# test.py can flip these before calling kernel() to get an NTFF profile.
TRACE = False
LAST_RESULT = None


def _build(has_bias: bool, c1: float, c2: float) -> bass.Bass:
    nc = bacc.Bacc("TRN2", target_bir_lowering=False)
    x = nc.dram_tensor("x", [ROWS, D], F16, kind="ExternalInput")
    wt = nc.dram_tensor("wt", [P, KCH, L], F16, kind="ExternalInput")
    if has_bias:
        b3 = nc.dram_tensor("b3", [1, D], F16, kind="ExternalInput")
    y = nc.dram_tensor("y", [ROWS, D], F16, kind="ExternalOutput")

    # row r = p*TILES + t  ->  [p][t][d]; consecutive t are consecutive DRAM
    # rows, so a [:, t0:t0+su, :] DMA moves su*2KiB contiguous per partition
    xv = x.rearrange("(p t) d -> p t d", t=TILES)
    yv = y.rearrange("(p t) d -> p t d", t=TILES)

    with tile.TileContext(nc) as tc, ExitStack() as ctx:
        singles = ctx.enter_context(tc.tile_pool(name="singles", bufs=1))
        xpool = ctx.enter_context(tc.tile_pool(name="xpool", bufs=4))
        opool = ctx.enter_context(tc.tile_pool(name="opool", bufs=3))
        xtpool = ctx.enter_context(tc.tile_pool(name="xtpool", bufs=4))
        small = ctx.enter_context(tc.tile_pool(name="small", bufs=4))
        psA = ctx.enter_context(tc.tile_pool(name="psA", bufs=4, space="PSUM"))
        psB = ctx.enter_context(tc.tile_pool(name="psB", bufs=3, space="PSUM"))

        # tiny constant DMA goes on the SWDGE ring so it cannot delay the
        # first big x in-DMA on the SP HWDGE ring
        wt_sb = singles.tile([P, KCH, L], F16)
        nc.gpsimd.dma_start(out=wt_sb, in_=wt[:])
        eye_sb = singles.tile([P, P], F16)
        make_identity(nc, eye_sb)
        if has_bias:
            b3_sb = singles.tile([P, D], F16)
            b3_bcast = bass.AP(
                tensor=b3.tensor, offset=b3.offset, ap=[[0, P], b3.ap[1]]
            )
            nc.gpsimd.dma_start(out=b3_sb, in_=b3_bcast)

        t_off = 0
        for s, su in enumerate(SCHED):
            xs = xpool.tile([P, su, D], F16, tag="xs")
            if s == 0:
                # split the first in-DMA so the PE can start on tile 0
                # after 256 KiB instead of waiting for the full 1 MiB
                nc.sync.dma_start(out=xs[:, 0:1, :], in_=xv[:, t_off : t_off + 1, :])
                nc.sync.dma_start(
                    out=xs[:, 1:su, :], in_=xv[:, t_off + 1 : t_off + su, :]
                )
            else:
                nc.sync.dma_start(out=xs, in_=xv[:, t_off : t_off + su, :])
            ys = opool.tile([P, su, D], F16, tag="ys")
            pt4 = psB.tile([P, su, L], F32)
            for u in range(su):
                # transpose x tile: 8 x [128,128] blocks -> psum
                pxt = psA.tile([P, KCH, P], F16)
                for k in range(KCH):
                    nc.tensor.transpose(
                        pxt[:, k, :], xs[:, u, k * P : (k + 1) * P], eye_sb
                    )
                # PSUM -> SBUF move, streamed as 32-bit f16 PAIRS so the
                # engines see 512 columns instead of 1024.  The ACT float
                # path would flush denormal f32 bit patterns, but the host
                # pre-nudges every |x| < 2^-17 element (~100 in 16M, error
                # < 2e-5) so no pair can encode a denormal.  The last tile
                # goes via DVE (int path) to balance the engines.
                xt = xtpool.tile([P, KCH, P], F16)
                if u == su - 1:
                    nc.vector.tensor_copy(xt.bitcast(U32), pxt.bitcast(U32))
                else:
                    nc.scalar.copy(out=xt.bitcast(F32), in_=pxt.bitcast(F32))
                # t[row, l] = sum_d x[row, d] * W[l, d], accumulated over chunks
                pt = pt4[:, u, :]
                for k in range(KCH):
                    nc.tensor.matmul(
                        pt,
                        xt[:, k, :],
                        wt_sb[:, k, :],
                        start=(k == 0),
                        stop=(k == KCH - 1),
                    )
            # a3 = ((1+t0)(1+t1)+c1)(1+t2)+c2, batched for the whole
            # supertile: 3 DVE ops instead of 3 per tile
            ut4 = small.tile([P, SUPER, L], F32, tag="ut")
            nc.vector.tensor_scalar_add(ut4[:, :su, :], pt4, 1.0)
            m4 = small.tile([P, SUPER], F32, tag="m4")
            nc.vector.tensor_mul(m4[:, :su], ut4[:, :su, 0], ut4[:, :su, 1])
            if c1 != 0.0:
                nc.vector.tensor_scalar_add(m4[:, :su], m4[:, :su], c1)
            a3_4 = small.tile([P, SUPER], F32, tag="a3")
            nc.vector.tensor_mul(a3_4[:, :su], m4[:, :su], ut4[:, :su, 2])
            if c2 != 0.0:
                nc.vector.tensor_scalar_add(a3_4[:, :su], a3_4[:, :su], c2)
            # out = x * a3 (+ B_L)
            for u in range(su):
                nc.vector.tensor_scalar_mul(
                    ys[:, u, :], xs[:, u, :], a3_4[:, u : u + 1]
                )
                if has_bias:
                    nc.vector.tensor_add(ys[:, u, :], ys[:, u, :], b3_sb)
            # out-DMAs issue on the Sync engine: it is idle once the five
            # in-DMAs are queued (all before the first store is ready), so
            # the pre-issue semaphore wait never stalls the ACT copy
            # stream.  The last supertile's store is split so the bulk of
            # it starts while the final tile is still being scaled.
            if s == len(SCHED) - 1 and su > 1:
                nc.sync.dma_start(
                    out=yv[:, t_off : t_off + su - 1, :], in_=ys[:, : su - 1, :]
                )
                nc.sync.dma_start(
                    out=yv[:, t_off + su - 1 : t_off + su, :], in_=ys[:, su - 1 :, :]
                )
            else:
                nc.sync.dma_start(out=yv[:, t_off : t_off + su, :], in_=ys)
            t_off += su
    nc.finalize()
    return nc


def kernel(x, W, bias):
    global LAST_RESULT
    x2 = np.asarray(x, dtype=np.float32).reshape(B, D)
    W2 = np.asarray(W, dtype=np.float32).reshape(L, D)
    B2 = np.asarray(bias, dtype=np.float32).reshape(L, D)

    # host-side constants
    has_bias = bool(np.any(B2 != 0.0))
    c1 = float(B2[0] @ W2[1])
    c2 = float((B2[0] + B2[1]) @ W2[2])
    b3_host = np.ascontiguousarray(B2.sum(axis=0).reshape(1, D).astype(np.float16))
    # wt[p, k, l] = W[l, k*128 + p]
    wt_host = np.ascontiguousarray(
        W2.T.reshape(KCH, P, L).transpose(1, 0, 2).astype(np.float16)
    )

    nc = _build(has_bias, c1 if has_bias else 0.0, c2 if has_bias else 0.0)

    x16 = np.ascontiguousarray(x2.astype(np.float16))
    # Nudge any |x| < 2^-17 up to +-2^-17 so no adjacent f16 pair can form
    # a denormal f32 bit pattern (the device copies the transposed tiles
    # through the ACT engine as f32 pairs, which flushes denormals).
    xb = x16.view(np.uint16)
    tiny = (xb & 0x7F80) == 0
    xb[tiny] = (xb[tiny] & 0x8000) | 0x0080
    shards = np.split(x16, N_CORES, axis=0)
    in_maps = []
    for c in range(N_CORES):
        m = {"x": shards[c], "wt": wt_host}
        if has_bias:
            m["b3"] = b3_host
        in_maps.append(m)

    kwargs = {}
    if TRACE:
        kwargs = dict(trace=True, trace_cores=[0])
    res = run_bass_kernel_spmd(nc, in_maps, core_ids=list(range(N_CORES)), **kwargs)
    LAST_RESULT = res
    out = np.concatenate(
        [res.results[c]["y"].astype(np.float32) for c in range(N_CORES)], axis=0
    )
    return np.ascontiguousarray(out.reshape(B, D, 1))


# revision 48
# speedup vs baseline: 1.2467x; 1.2467x over previous
# DCN CrossLayer kernel for Trainium2 (8 NeuronCores, data-parallel over batch).
#
# Reference computation (per example row x of length D, L=3 layers):
#   cross = x
#   for i in range(L):
#       s_i   = <cross, W_i>                  (scalar per example)
#       cross = x * s_i + bias_i + cross
#
# Algebraic collapse: cross_i = a_i * x + B_i with per-example scalar a_i and
# batch-independent vector B_i = sum_{j<i} bias_j.  Then
#   s_i     = a_i * t_i + c_i,   t_i = <x, W_i>,  c_i = <B_i, W_i>
#   a_{i+1} = a_i * (1 + t_i) + c_i
#   out     = a_L * x + B_L
# so the device kernel only needs the three dot products t_i = <x, W_i>
# (one skinny matmul against W^T), a tiny per-row recurrence, and one
# per-row scale of x.  c_i and B_L are computed on the host (they do not
# depend on the batch).
#
# The kernel is HBM-bandwidth-bound (~400 GB/s aggregate per core across the
# 16 SDMA engines).  To halve the traffic the device I/O is fp16: the host
# casts x -> f16 before upload and upcasts y f16 -> f32 after download.  The
# dot products already ran in f16 on the PE (error ~5e-4, gate is 2e-2).
#
# Device plan per core (2048 rows of 1024, f16):
#   - rows mapped p-major (row = p*TILES + t) so each partition's DMA run is
#     su contiguous rows (8 KiB at su=4) instead of one row (2 KiB)
#   - DMA x in supertiles [128 part, 4, 1024] f16 on the SP HWDGE ring
#   - PE transposes each [128,128] block of x -> PSUM, ACT copies to SBUF
#   - PE matmuls xt_k^T @ Wt_k accumulating t [128 rows, 3] in PSUM
#   - DVE: a3 = ((1+t0)(1+t1)+c1)(1+t2)+c2 ; y = x * a3 (+ B_L), all f16
#   - DMA y out on the ACT HWDGE ring so it can't FIFO-block in-DMAs
import os
from contextlib import ExitStack

import numpy as np

import concourse.bacc as bacc
import concourse.bass as bass
import concourse.tile as tile
from concourse import mybir
from concourse.bass_utils import run_bass_kernel_spmd
from concourse.masks import make_identity

B, D, L = 16384, 1024, 3
N_CORES = 8
ROWS = B // N_CORES  # rows per core
P = 128
TILES = ROWS // P  # 16 row-tiles per core
SUPER = 4  # row-tiles per supertile (1 MiB f16 DMA)
SCHED = [SUPER] * (TILES // SUPER)
KCH = D // P  # 8 d-chunks of 128

F32 = mybir.dt.float32
F16 = mybir.dt.float16
U32 = mybir.dt.uint32

# test.py can flip these before calling kernel() to get an NTFF profile.
TRACE = False
LAST_RESULT = None


def _build(has_bias: bool, c1: float, c2: float) -> bass.Bass:
    nc = bacc.Bacc("TRN2", target_bir_lowering=False)
    x = nc.dram_tensor("x", [ROWS, D], F16, kind="ExternalInput")
    wt = nc.dram_tensor("wt", [P, KCH, L], F16, kind="ExternalInput")
    if has_bias:
        b3 = nc.dram_tensor("b3", [1, D], F16, kind="ExternalInput")
    y = nc.dram_tensor("y", [ROWS, D], F16, kind="ExternalOutput")

    # row r = p*TILES + t  ->  [p][t][d]; consecutive t are consecutive DRAM
    # rows, so a [:, t0:t0+su, :] DMA moves su*2KiB contiguous per partition
    xv = x.rearrange("(p t) d -> p t d", t=TILES)
    yv = y.rearrange("(p t) d -> p t d", t=TILES)

    with tile.TileContext(nc) as tc, ExitStack() as ctx:
        singles = ctx.enter_context(tc.tile_pool(name="singles", bufs=1))
        xpool = ctx.enter_context(tc.tile_pool(name="xpool", bufs=4))
        opool = ctx.enter_context(tc.tile_pool(name="opool", bufs=3))
        xtpool = ctx.enter_context(tc.tile_pool(name="xtpool", bufs=4))
        small = ctx.enter_context(tc.tile_pool(name="small", bufs=4))
        psA = ctx.enter_context(tc.tile_pool(name="psA", bufs=4, space="PSUM"))
        psB = ctx.enter_context(tc.tile_pool(name="psB", bufs=3, space="PSUM"))

        # tiny constant DMA goes on the SWDGE ring so it cannot delay the
        # first big x in-DMA on the SP HWDGE ring
        wt_sb = singles.tile([P, KCH, L], F16)
        nc.gpsimd.dma_start(out=wt_sb, in_=wt[:])
        eye_sb = singles.tile([P, P], F16)
        make_identity(nc, eye_sb)
        if has_bias:
            b3_sb = singles.tile([P, D], F16)
            b3_bcast = bass.AP(
                tensor=b3.tensor, offset=b3.offset, ap=[[0, P], b3.ap[1]]
            )
            nc.gpsimd.dma_start(out=b3_sb, in_=b3_bcast)

        t_off = 0
        for s, su in enumerate(SCHED):
            xs = xpool.tile([P, su, D], F16, tag="xs")
            if s == 0:
                # split the first in-DMA so the PE can start on tile 0
                # after 256 KiB instead of waiting for the full 1 MiB
                nc.sync.dma_start(out=xs[:, 0:1, :], in_=xv[:, t_off : t_off + 1, :])
                nc.sync.dma_start(
                    out=xs[:, 1:su, :], in_=xv[:, t_off + 1 : t_off + su, :]
                )
            else:
                nc.sync.dma_start(out=xs, in_=xv[:, t_off : t_off + su, :])
            ys = opool.tile([P, su, D], F16, tag="ys")
            pt4 = psB.tile([P, su, L], F32)
            for u in range(su):
                # transpose x tile: 8 x [128,128] blocks -> psum
                pxt = psA.tile([P, KCH, P], F16)
                for k in range(KCH):
                    nc.tensor.transpose(
                        pxt[:, k, :], xs[:, u, k * P : (k + 1) * P], eye_sb
                    )
                # PSUM -> SBUF move; the supertile's LAST tile goes via DVE
                # as uint32 (f16 pairs, bit-exact int path, 0.69us vs
                # 1.11us): at u3 the previous supertile's ymul burst has
                # already drained from the DVE queue, so the copy does not
                # delay the dots behind it
                xt = xtpool.tile([P, KCH, P], F16)
                if u == su - 1:
                    nc.vector.tensor_copy(xt.bitcast(U32), pxt.bitcast(U32))
                else:
                    nc.scalar.copy(out=xt, in_=pxt)
                # t[row, l] = sum_d x[row, d] * W[l, d], accumulated over chunks
                pt = pt4[:, u, :]
                for k in range(KCH):
                    nc.tensor.matmul(
                        pt,
                        xt[:, k, :],
                        wt_sb[:, k, :],
                        start=(k == 0),
                        stop=(k == KCH - 1),
                    )
            # a3 = ((1+t0)(1+t1)+c1)(1+t2)+c2, batched for the whole
            # supertile: 3 DVE ops instead of 3 per tile
            ut4 = small.tile([P, SUPER, L], F32, tag="ut")
            nc.vector.tensor_scalar_add(ut4[:, :su, :], pt4, 1.0)
            m4 = small.tile([P, SUPER], F32, tag="m4")
            nc.vector.tensor_mul(m4[:, :su], ut4[:, :su, 0], ut4[:, :su, 1])
            if c1 != 0.0:
                nc.vector.tensor_scalar_add(m4[:, :su], m4[:, :su], c1)
            a3_4 = small.tile([P, SUPER], F32, tag="a3")
            nc.vector.tensor_mul(a3_4[:, :su], m4[:, :su], ut4[:, :su, 2])
            if c2 != 0.0:
                nc.vector.tensor_scalar_add(a3_4[:, :su], a3_4[:, :su], c2)
            # out = x * a3 (+ B_L)
            for u in range(su):
                nc.vector.tensor_scalar_mul(
                    ys[:, u, :], xs[:, u, :], a3_4[:, u : u + 1]
                )
                if has_bias:
                    nc.vector.tensor_add(ys[:, u, :], ys[:, u, :], b3_sb)
            # out-DMAs issue on the Sync engine: it is idle once the five
            # in-DMAs are queued (all before the first store is ready), so
            # the pre-issue semaphore wait never stalls the ACT copy
            # stream.  The last supertile's store is split so the bulk of
            # it starts while the final tile is still being scaled.
            if s == len(SCHED) - 1 and su > 1:
                nc.sync.dma_start(
                    out=yv[:, t_off : t_off + su - 1, :], in_=ys[:, : su - 1, :]
                )
                nc.sync.dma_start(
                    out=yv[:, t_off + su - 1 : t_off + su, :], in_=ys[:, su - 1 :, :]
                )
            else:
                nc.sync.dma_start(out=yv[:, t_off : t_off + su, :], in_=ys)
            t_off += su
    nc.finalize()
    return nc


def kernel(x, W, bias):
    global LAST_RESULT
    x2 = np.asarray(x, dtype=np.float32).reshape(B, D)
    W2 = np.asarray(W, dtype=np.float32).reshape(L, D)
    B2 = np.asarray(bias, dtype=np.float32).reshape(L, D)

    # host-side constants
    has_bias = bool(np.any(B2 != 0.0))
    c1 = float(B2[0] @ W2[1])
    c2 = float((B2[0] + B2[1]) @ W2[2])
    b3_host = np.ascontiguousarray(B2.sum(axis=0).reshape(1, D).astype(np.float16))
    # wt[p, k, l] = W[l, k*128 + p]
    wt_host = np.ascontiguousarray(
        W2.T.reshape(KCH, P, L).transpose(1, 0, 2).astype(np.float16)
    )

    nc = _build(has_bias, c1 if has_bias else 0.0, c2 if has_bias else 0.0)

    x16 = np.ascontiguousarray(x2.astype(np.float16))
    shards = np.split(x16, N_CORES, axis=0)
    in_maps = []
    for c in range(N_CORES):
        m = {"x": shards[c], "wt": wt_host}
        if has_bias:
            m["b3"] = b3_host
        in_maps.append(m)

    kwargs = {}
    if TRACE:
        kwargs = dict(trace=True, trace_cores=[0])
    res = run_bass_kernel_spmd(nc, in_maps, core_ids=list(range(N_CORES)), **kwargs)
    LAST_RESULT = res
    out = np.concatenate(
        [res.results[c]["y"].astype(np.float32) for c in range(N_CORES)], axis=0
    )
    return np.ascontiguousarray(out.reshape(B, D, 1))


# revision 49
# speedup vs baseline: 1.2674x; 1.0166x over previous
# DCN CrossLayer kernel for Trainium2 (8 NeuronCores, data-parallel over batch).
#
# Reference computation (per example row x of length D, L=3 layers):
#   cross = x
#   for i in range(L):
#       s_i   = <cross, W_i>                  (scalar per example)
#       cross = x * s_i + bias_i + cross
#
# Algebraic collapse: cross_i = a_i * x + B_i with per-example scalar a_i and
# batch-independent vector B_i = sum_{j<i} bias_j.  Then
#   s_i     = a_i * t_i + c_i,   t_i = <x, W_i>,  c_i = <B_i, W_i>
#   a_{i+1} = a_i * (1 + t_i) + c_i
#   out     = a_L * x + B_L
# so the device kernel only needs the three dot products t_i = <x, W_i>
# (one skinny matmul against W^T), a tiny per-row recurrence, and one
# per-row scale of x.  c_i and B_L are computed on the host (they do not
# depend on the batch).
#
# The kernel is HBM-bandwidth-bound (~400 GB/s aggregate per core across the
# 16 SDMA engines).  To halve the traffic the device I/O is fp16: the host
# casts x -> f16 before upload and upcasts y f16 -> f32 after download.  The
# dot products already ran in f16 on the PE (error ~5e-4, gate is 2e-2).
#
# Device plan per core (2048 rows of 1024, f16):
#   - rows mapped p-major (row = p*TILES + t) so each partition's DMA run is
#     su contiguous rows (8 KiB at su=4) instead of one row (2 KiB)
#   - DMA x in supertiles [128 part, 4, 1024] f16 on the SP HWDGE ring
#   - PE transposes each [128,128] block of x -> PSUM, ACT copies to SBUF
#   - PE matmuls xt_k^T @ Wt_k accumulating t [128 rows, 3] in PSUM
#   - DVE: a3 = ((1+t0)(1+t1)+c1)(1+t2)+c2 ; y = x * a3 (+ B_L), all f16
#   - DMA y out on the ACT HWDGE ring so it can't FIFO-block in-DMAs
import os
from contextlib import ExitStack

import numpy as np

import concourse.bacc as bacc
import concourse.bass as bass
import concourse.tile as tile
from concourse import mybir
from concourse.bass_utils import run_bass_kernel_spmd
from concourse.masks import make_identity

B, D, L = 16384, 1024, 3
N_CORES = 8
ROWS = B // N_CORES  # rows per core
P = 128
TILES = ROWS // P  # 16 row-tiles per core
SUPER = 4  # row-tiles per supertile (1 MiB f16 DMA)
SCHED = [SUPER] * (TILES // SUPER)
KCH = D // P  # 8 d-chunks of 128

F32 = mybir.dt.float32
F16 = mybir.dt.float16
U32 = mybir.dt.uint32

# test.py can flip these before calling kernel() to get an NTFF profile.
TRACE = False
LAST_RESULT = None


def _build(has_bias: bool, c1: float, c2: float) -> bass.Bass:
    nc = bacc.Bacc("TRN2", target_bir_lowering=False)
    x = nc.dram_tensor("x", [ROWS, D], F16, kind="ExternalInput")
    wt = nc.dram_tensor("wt", [P, KCH, L], F16, kind="ExternalInput")
    if has_bias:
        b3 = nc.dram_tensor("b3", [1, D], F16, kind="ExternalInput")
    y = nc.dram_tensor("y", [ROWS, D], F16, kind="ExternalOutput")

    # row r = p*TILES + t  ->  [p][t][d]; consecutive t are consecutive DRAM
    # rows, so a [:, t0:t0+su, :] DMA moves su*2KiB contiguous per partition
    xv = x.rearrange("(p t) d -> p t d", t=TILES)
    yv = y.rearrange("(p t) d -> p t d", t=TILES)

    with tile.TileContext(nc) as tc, ExitStack() as ctx:
        singles = ctx.enter_context(tc.tile_pool(name="singles", bufs=1))
        xpool = ctx.enter_context(tc.tile_pool(name="xpool", bufs=4))
        opool = ctx.enter_context(tc.tile_pool(name="opool", bufs=3))
        xtpool = ctx.enter_context(tc.tile_pool(name="xtpool", bufs=4))
        small = ctx.enter_context(tc.tile_pool(name="small", bufs=4))
        psA = ctx.enter_context(tc.tile_pool(name="psA", bufs=4, space="PSUM"))
        psB = ctx.enter_context(tc.tile_pool(name="psB", bufs=3, space="PSUM"))

        # tiny constant DMA goes on the SWDGE ring so it cannot delay the
        # first big x in-DMA on the SP HWDGE ring
        wt_sb = singles.tile([P, KCH, L], F16)
        nc.gpsimd.dma_start(out=wt_sb, in_=wt[:])
        eye_sb = singles.tile([P, P], F16)
        make_identity(nc, eye_sb)
        if has_bias:
            b3_sb = singles.tile([P, D], F16)
            b3_bcast = bass.AP(
                tensor=b3.tensor, offset=b3.offset, ap=[[0, P], b3.ap[1]]
            )
            nc.gpsimd.dma_start(out=b3_sb, in_=b3_bcast)

        # The last tile's dots (and everything downstream of them) are
        # deferred into the next supertile, after its first transposes:
        # otherwise the in-order PE queue stalls at every supertile
        # boundary waiting for the last PSUM->SBUF copy.
        def flush(p, final):
            pt4, fxs, fys, fsu, off, fxt = p
            for k in range(KCH):
                nc.tensor.matmul(
                    pt4[:, fsu - 1, :],
                    fxt[:, k, :],
                    wt_sb[:, k, :],
                    start=(k == 0),
                    stop=(k == KCH - 1),
                )
            # a3 = ((1+t0)(1+t1)+c1)(1+t2)+c2, batched for the whole
            # supertile: 3 DVE ops instead of 3 per tile
            ut4 = small.tile([P, SUPER, L], F32, tag="ut")
            nc.vector.tensor_scalar_add(ut4[:, :fsu, :], pt4, 1.0)
            m4 = small.tile([P, SUPER], F32, tag="m4")
            nc.vector.tensor_mul(m4[:, :fsu], ut4[:, :fsu, 0], ut4[:, :fsu, 1])
            if c1 != 0.0:
                nc.vector.tensor_scalar_add(m4[:, :fsu], m4[:, :fsu], c1)
            a3_4 = small.tile([P, SUPER], F32, tag="a3")
            nc.vector.tensor_mul(a3_4[:, :fsu], m4[:, :fsu], ut4[:, :fsu, 2])
            if c2 != 0.0:
                nc.vector.tensor_scalar_add(a3_4[:, :fsu], a3_4[:, :fsu], c2)
            # out = x * a3 (+ B_L)
            for fu in range(fsu):
                nc.vector.tensor_scalar_mul(
                    fys[:, fu, :], fxs[:, fu, :], a3_4[:, fu : fu + 1]
                )
                if has_bias:
                    nc.vector.tensor_add(fys[:, fu, :], fys[:, fu, :], b3_sb)
            # out-DMAs issue on the Sync engine: it is idle once the five
            # in-DMAs are queued, so the pre-issue semaphore wait never
            # stalls the ACT copy stream.  The very last store is split so
            # its bulk starts while the final tile is still being scaled.
            if final and fsu > 1:
                nc.sync.dma_start(
                    out=yv[:, off : off + fsu - 1, :], in_=fys[:, : fsu - 1, :]
                )
                nc.sync.dma_start(
                    out=yv[:, off + fsu - 1 : off + fsu, :], in_=fys[:, fsu - 1 :, :]
                )
            else:
                nc.sync.dma_start(out=yv[:, off : off + fsu, :], in_=fys)

        t_off = 0
        prev = None  # deferred tail work of the previous supertile
        for s, su in enumerate(SCHED):
            xs = xpool.tile([P, su, D], F16, tag="xs")
            if s == 0:
                # split the first in-DMA so the PE can start on tile 0
                # after 256 KiB instead of waiting for the full 1 MiB
                nc.sync.dma_start(out=xs[:, 0:1, :], in_=xv[:, t_off : t_off + 1, :])
                nc.sync.dma_start(
                    out=xs[:, 1:su, :], in_=xv[:, t_off + 1 : t_off + su, :]
                )
            else:
                nc.sync.dma_start(out=xs, in_=xv[:, t_off : t_off + su, :])
            ys = opool.tile([P, su, D], F16, tag="ys")
            pt4 = psB.tile([P, su, L], F32)
            xt_last = None
            for u in range(su):
                # transpose x tile: 8 x [128,128] blocks -> psum
                pxt = psA.tile([P, KCH, P], F16)
                for k in range(KCH):
                    nc.tensor.transpose(
                        pxt[:, k, :], xs[:, u, k * P : (k + 1) * P], eye_sb
                    )
                # PSUM -> SBUF move; the supertile's LAST tile goes via DVE
                # as uint32 (f16 pairs, bit-exact int path, 0.69us vs
                # 1.11us): its dots are deferred anyway, and the previous
                # ymul burst has drained from the DVE queue by then
                xt = xtpool.tile([P, KCH, P], F16)
                if u == su - 1:
                    nc.vector.tensor_copy(xt.bitcast(U32), pxt.bitcast(U32))
                else:
                    nc.scalar.copy(out=xt, in_=pxt)
                if u == 0 and prev is not None:
                    flush(prev, final=False)
                    prev = None
                if u < su - 1:
                    # t[row, l] = sum_d x[row, d]*W[l, d] over the 8 chunks
                    pt = pt4[:, u, :]
                    for k in range(KCH):
                        nc.tensor.matmul(
                            pt,
                            xt[:, k, :],
                            wt_sb[:, k, :],
                            start=(k == 0),
                            stop=(k == KCH - 1),
                        )
                else:
                    xt_last = xt
            prev = (pt4, xs, ys, su, t_off, xt_last)
            t_off += su
        flush(prev, final=True)
    nc.finalize()
    return nc


def kernel(x, W, bias):
    global LAST_RESULT
    x2 = np.asarray(x, dtype=np.float32).reshape(B, D)
    W2 = np.asarray(W, dtype=np.float32).reshape(L, D)
    B2 = np.asarray(bias, dtype=np.float32).reshape(L, D)

    # host-side constants
    has_bias = bool(np.any(B2 != 0.0))
    c1 = float(B2[0] @ W2[1])
    c2 = float((B2[0] + B2[1]) @ W2[2])
    b3_host = np.ascontiguousarray(B2.sum(axis=0).reshape(1, D).astype(np.float16))
    # wt[p, k, l] = W[l, k*128 + p]
    wt_host = np.ascontiguousarray(
        W2.T.reshape(KCH, P, L).transpose(1, 0, 2).astype(np.float16)
    )

    nc = _build(has_bias, c1 if has_bias else 0.0, c2 if has_bias else 0.0)

    x16 = np.ascontiguousarray(x2.astype(np.float16))
    shards = np.split(x16, N_CORES, axis=0)
    in_maps = []
    for c in range(N_CORES):
        m = {"x": shards[c], "wt": wt_host}
        if has_bias:
            m["b3"] = b3_host
        in_maps.append(m)

    kwargs = {}
    if TRACE:
        kwargs = dict(trace=True, trace_cores=[0])
    res = run_bass_kernel_spmd(nc, in_maps, core_ids=list(range(N_CORES)), **kwargs)
    LAST_RESULT = res
    out = np.concatenate(
        [res.results[c]["y"].astype(np.float32) for c in range(N_CORES)], axis=0
    )
    return np.ascontiguousarray(out.reshape(B, D, 1))
